# revision 1
# baseline (speedup 1.0000x reference)
"""Trainium2 Bass kernel for nn_Attn_19464791785826.

Reference computation (per batch b of 32):
    proj[l, :] = enc[b, l] @ W.T + bias            # [4096, 512]
    energies[l] = hidden[b] . proj[l]              # [4096]
    out[b, 0, :] = softmax(energies)               # [4096]

Key algebraic rewrite: energies[l] = (hidden[b] @ W) . enc[b, l] + hidden[b].bias.
The bias term is constant across l, so softmax cancels it exactly. The kernel
therefore computes q = hidden @ W on device (tiny), then a mat-vec against the
256 MiB encoder_outputs tensor (the memory-bound part), then a softmax.

Sharding: data-parallel over batch. 32 batches / 8 cores = 4 batches per core.
W replicated. No collectives; the host gathers the per-core [4, 4096] outputs
and undoes an on-chip layout permutation (part of unsharding).

Per-core dataflow:
  - setup: hid transposed in 128-blocks on PE; for each batch, the hid column
    is replicated along free dims on DVE (tensor_scalar vs ones) and fed as a
    [g,128] stationary to PE so the q = hid @ W result lands already
    partition-replicated ([128, H] per batch) -- no DRAM bounce needed.
  - main loop (per batch, per 2 MiB l-chunk): flat contiguous DMA, so SBUF
    partition p holds l = c*CL + p*tpc + i (16 KiB contiguous descriptors).
    One batched DVE multiply per chunk against the replicated q (0-stride AP);
    the row-sum over h runs either as a single 3D DVE tensor_reduce or as
    per-subtile ScalarE Copy-with-accumulate ops, statically load-balanced
    (1/4 of chunks on DVE, 3/4 on ACT).
  - softmax per batch over the [128, ncols] energy tile: free-dim max, PE
    transpose + reduce for the cross-partition max, ScalarE exp with fused
    per-partition sum, ones-matmul for the cross-partition sum, reciprocal,
    PE transpose to [ncols, 128] with normalization fused into the PSUM->SBUF
    evacuation, contiguous DMA out.
"""

import numpy as np

import concourse.bass as bass
from concourse import bacc
import concourse.mybir as mybir
import concourse.tile as tile
from concourse.bass_utils import run_bass_kernel_spmd
from concourse.masks import make_identity

H = 512
L = 4096
B = 32
N_CORES = 8
BPC = B // N_CORES  # batches per core
CHUNK_L = 1024

F32 = mybir.dt.float32

# the last chunk of each batch reduces on DVE (single fast op feeding the
# softmax max-reduce on the same engine); earlier chunks reduce on ACT


def emit_core_kernel(nc, tc, enc, hid, w, out, bpc, l_total, chunk_l):
    """Emit the per-core kernel into an open TileContext."""
    n_chunks = l_total // chunk_l
    tpc = chunk_l // 128          # l-subtiles per chunk
    ncols = l_total // 128        # energy columns per batch
    kblk = H // 128               # 128-blocks of the contraction dim

    import contextlib
    ctx = contextlib.ExitStack()
    with ctx:
        const = ctx.enter_context(tc.tile_pool(name="const", bufs=1))
        setup = ctx.enter_context(tc.tile_pool(name="setup", bufs=1))
        encp = ctx.enter_context(tc.tile_pool(name="encp", bufs=5))
        scr = ctx.enter_context(tc.tile_pool(name="scr", bufs=4))
        epool = ctx.enter_context(tc.tile_pool(name="epool", bufs=2))
        small = ctx.enter_context(tc.tile_pool(name="small", bufs=2))
        opool = ctx.enter_context(tc.tile_pool(name="opool", bufs=2))
        psp = ctx.enter_context(tc.tile_pool(name="psp", bufs=2, space="PSUM"))
        ptp = ctx.enter_context(tc.tile_pool(name="ptp", bufs=2, space="PSUM"))
        pss = ctx.enter_context(tc.tile_pool(name="pss", bufs=4, space="PSUM"))

        # ---- constants -------------------------------------------------
        ident = const.tile([128, 128], F32)
        make_identity(nc, ident)
        ones_sq = const.tile([128, 128], F32)
        nc.vector.memset(ones_sq, 1.0)
        ones_row = const.tile([1, 128], F32)
        nc.vector.memset(ones_row, 1.0)
        neg_ones_row = const.tile([1, 128], F32)
        nc.vector.memset(neg_ones_row, -1.0)
        ones_col = const.tile([128, 1], F32)
        nc.vector.memset(ones_col, 1.0)

        # preload the Exp table so batch 0's softmax doesn't stall on it
        dexp = small.tile([1, 1], F32, tag="dexp")
        nc.scalar.activation(dexp, ones_row[:1, :1],
                             mybir.ActivationFunctionType.Exp)

        # ---- setup: qb[b] = hid[b] @ W, replicated across partitions ---
        hid_sb = setup.tile([bpc, H], F32)
        nc.sync.dma_start(out=hid_sb, in_=hid[:, :])
        w_sb = setup.tile([128, kblk, H], F32)  # w_sb[g, k, h] = W[k*128+g, h]
        nc.sync.dma_start(out=w_sb, in_=w.rearrange("(k g) h -> g k h", g=128))

        hid_t = setup.tile([128, kblk, bpc], F32)  # hid_t[g, k, b] = hid[b, k*128+g]
        for k in range(kblk):
            tps = pss.tile([128, bpc], F32, tag="sp")
            nc.tensor.transpose(tps, hid_sb[:, k * 128:(k + 1) * 128],
                                ident[:bpc, :bpc])
            nc.scalar.copy(hid_t[:, k, :], tps)

        # qb[:, b, h] = sum_g hid[b, g] W[g, h] for every partition: feed PE a
        # column-replicated hid block as the stationary operand.
        qb = setup.tile([128, bpc, H], F32)
        for b in range(bpc):
            hrep = setup.tile([128, kblk, 128], F32, tag="hrep")
            for k in range(kblk):
                nc.vector.tensor_scalar_mul(hrep[:, k, :], ones_sq,
                                            hid_t[:, k, b:b + 1])
            qb_ps = psp.tile([128, H], F32, tag="bank")
            for k in range(kblk):
                nc.tensor.matmul(qb_ps, lhsT=hrep[:, k, :], rhs=w_sb[:, k, :],
                                 start=(k == 0), stop=(k == kblk - 1))
            nc.scalar.copy(qb[:, b, :], qb_ps)

        # ---- main loop -------------------------------------------------
        for b in range(bpc):
            eb = epool.tile([128, ncols], F32)  # eb[p, c*tpc+i] = E[c*CL + p*tpc + i]
            for c in range(n_chunks):
                et = encp.tile([128, tpc, H], F32)
                nc.sync.dma_start(
                    out=et,
                    in_=enc[b, c * chunk_l:(c + 1) * chunk_l, :]
                        .rearrange("(p i) h -> p i h", p=128),
                )
                # one batched multiply per chunk; q[b] broadcast over the
                # l-subtile dim with a 0-stride AP
                prod = scr.tile([128, tpc, H], F32)
                qv = qb[:, b, :]
                q_bc = bass.AP(tensor=qv.tensor, offset=qv.offset,
                               ap=[qv.ap[0], [0, tpc], qv.ap[1]])
                nc.vector.tensor_mul(prod, et, q_bc)
                # row-sum over h, statically balanced between DVE and ACT
                if c == n_chunks - 1:
                    nc.vector.tensor_reduce(
                        eb[:, c * tpc:(c + 1) * tpc], prod,
                        axis=mybir.AxisListType.X, op=mybir.AluOpType.add)
                else:
                    for i in range(tpc):
                        col = c * tpc + i
                        junk = scr.tile([128, H], F32, tag="junk")
                        nc.scalar.activation(junk, prod[:, i, :],
                                             mybir.ActivationFunctionType.Copy,
                                             accum_out=eb[:, col:col + 1])

            # ---- softmax over the [128, ncols] energy tile -------------
            mp = small.tile([128, 1], F32)
            nc.vector.tensor_reduce(mp, eb, axis=mybir.AxisListType.X,
                                    op=mybir.AluOpType.max)
            mt_ps = pss.tile([1, 128], F32, tag="sp")
            nc.tensor.transpose(mt_ps, mp, ident)
            mt = small.tile([1, 128], F32)
            nc.scalar.copy(mt, mt_ps)
            mg = small.tile([1, 1], F32)
            nc.vector.tensor_reduce(mg, mt, axis=mybir.AxisListType.X,
                                    op=mybir.AluOpType.max)
            # broadcast -max to all partitions
            nm_ps = pss.tile([128, 1], F32, tag="sp")
            nc.tensor.matmul(nm_ps, lhsT=neg_ones_row, rhs=mg,
                             start=True, stop=True)
            negmax = small.tile([128, 1], F32)
            nc.scalar.copy(negmax, nm_ps)
            # exp(e - max) with fused per-partition sum
            pb = epool.tile([128, ncols], F32, tag="pb")
            sp_t = small.tile([128, 1], F32)
            nc.scalar.activation(pb, eb, mybir.ActivationFunctionType.Exp,
                                 bias=negmax, scale=1.0, accum_out=sp_t)
            # cross-partition sum -> total, then 1/total broadcast
            tot_ps = pss.tile([1, 1], F32, tag="sp")
            nc.tensor.matmul(tot_ps, lhsT=sp_t, rhs=ones_col,
                             start=True, stop=True)
            rec = small.tile([1, 1], F32)
            nc.vector.reciprocal(rec, tot_ps)
            rb_ps = pss.tile([128, 1], F32, tag="sp")
            nc.tensor.matmul(rb_ps, lhsT=ones_row, rhs=rec,
                             start=True, stop=True)
            rbc = small.tile([128, 1], F32)
            nc.scalar.copy(rbc, rb_ps)
            # transpose to [ncols, 128]; normalize on the PSUM->SBUF copy
            pt_ps = ptp.tile([ncols, 128], F32, tag="pt")
            nc.tensor.transpose(pt_ps, pb, ident)
            ob = opool.tile([ncols, 128], F32)
            nc.vector.tensor_scalar_mul(ob, pt_ps, rbc[:ncols, :])
            nc.sync.dma_start(out=out[b].rearrange("(t p) -> t p", p=128),
                              in_=ob)


def unpermute(out2d, l_total=L, chunk_l=CHUNK_L):
    """Undo the on-chip l-layout: device out[b, (c*tpc+i)*128 + p] holds
    prob(l = c*chunk_l + p*tpc + i)."""
    nb = out2d.shape[0]
    n_chunks = l_total // chunk_l
    tpc = chunk_l // 128
    return (out2d.reshape(nb, n_chunks, tpc, 128)
                 .transpose(0, 1, 3, 2)
                 .reshape(nb, l_total))


def build_bass(bpc=BPC, l_total=L, chunk_l=CHUNK_L):
    nc = bacc.Bacc(None)
    enc = nc.declare_dram_parameter("enc", [bpc, l_total, H], F32, isOutput=False)
    hid = nc.declare_dram_parameter("hid", [bpc, H], F32, isOutput=False)
    w = nc.declare_dram_parameter("w", [H, H], F32, isOutput=False)
    out = nc.declare_dram_parameter("out", [bpc, l_total], F32, isOutput=True)
    with tile.TileContext(nc) as tc:
        emit_core_kernel(nc, tc, enc, hid, w, out, bpc, l_total, chunk_l)
    nc.compile()
    return nc


_NC_CACHE = {}


def kernel(hidden, encoder_outputs, W, b):
    hidden = np.asarray(hidden, dtype=np.float32)
    encoder_outputs = np.asarray(encoder_outputs, dtype=np.float32)
    W = np.asarray(W, dtype=np.float32)
    # b only shifts every energy in a batch by a constant; softmax cancels it.

    key = "full"
    if key not in _NC_CACHE:
        _NC_CACHE[key] = build_bass()
    nc = _NC_CACHE[key]

    in_maps = []
    for c in range(N_CORES):
        sl = slice(c * BPC, (c + 1) * BPC)
        in_maps.append({
            "enc": np.ascontiguousarray(encoder_outputs[sl]),
            "hid": np.ascontiguousarray(hidden[0, sl]),
            "w": W,
        })
    results = run_bass_kernel_spmd(nc, in_maps, list(range(N_CORES))).results
    out = np.concatenate([r["out"] for r in results], axis=0)  # [32, 4096]
    out = unpermute(out)
    return out[:, None, :].astype(np.float32)

